# revision 24
# baseline (speedup 1.0000x reference)
"""Distributed Trainium2 kernel for causal attention with relative position
embeddings (Music-Transformer skew), TP over heads: 8 cores x 2 heads.

Reference computation (B=1, S=2048, H=1024, NH=16, D=64):
  qkv = x @ W_attn + b_attn ; q,k,v split into heads
  w   = (q k^T + skew(q E^T)) / sqrt(D) + causal_mask * -1e9
  a   = softmax(w); out = (a v, merged) @ W_proj + b_proj
  present = stack([k, v])

Sharding: core c owns heads (2c, 2c+1). Everything head-local on device;
host sums the 8 partial projection outputs (the "all-reduce") and
assembles `present` from per-core k/v shards.

Skew trick on device: rel[n, j] = q[n] . E[s-1-n+j] (valid j<=n).  Per
128-row q-block at n0 we matmul P[r, w] = q[n0+r] . E[mlo+w] with
mlo = S-128-n0 over a window W = n0+256 (E zero-padded to 2176 cols so
masked/out-of-range reads are in-bounds), write P to a DRAM bounce
buffer, and read it back through a diagonal access pattern
addr = (127+kc) + r*(W-1) + c which lands rel[r, kc+c] exactly.  The
rel tile is then accumulated into the scores PSUM with an
identity-weight matmul.  The causal mask is free: the last 128 columns
of each P buffer are read exclusively by masked positions, so they are
memset to -1e9 before the bounce.  exp() runs on ScalarE with
accum_out producing the softmax denominators for free; rows of A are
normalized in place with a per-partition scalar multiply.  A^T tiles
(TensorE transpose) feed (A^T)'V matmuls that run once per k-tile
across 4 strided q-blocks (N<=512, right-aligned), and the projection
consumes the avT accumulator directly.
"""

import numpy as np
import ml_dtypes

S = 2048
H = 1024
NH = 16
D = 64
HPC = 2          # heads per core
NCORES = 8
QB = 128         # q-block rows
EPAD = 2176      # padded E length: max m index = 2174
SCALE = 1.0 / 8.0  # 1/sqrt(D)
NEG = -1.0e9

BF16 = ml_dtypes.bfloat16


def _build_graph():
    import concourse.bacc as bacc
    import concourse.bass as bass
    import concourse.mybir as mybir
    import concourse.tile as tile

    fp32 = mybir.dt.float32
    bf16 = mybir.dt.bfloat16
    AF = mybir.ActivationFunctionType

    nc = bacc.Bacc(
        "TRN2",
        target_bir_lowering=False,
        debug=False,
        enable_asserts=False,
        num_devices=NCORES,
    )

    # ---- I/O ----
    xT = nc.dram_tensor("xT", [H, S], bf16, kind="ExternalInput")
    wqkv = nc.dram_tensor("wqkv", [H, 384], bf16, kind="ExternalInput")
    et = nc.dram_tensor("et", [128, EPAD], bf16, kind="ExternalInput")
    wp = nc.dram_tensor("wp", [128, H], bf16, kind="ExternalInput")
    ident_in = nc.dram_tensor("ident", [128, 128], bf16, kind="ExternalInput")

    out_p = nc.dram_tensor("out_p", [S, H], bf16, kind="ExternalOutput")
    kt_out = nc.dram_tensor("kt_out", [128, S], bf16, kind="ExternalOutput")
    v_out = nc.dram_tensor("v_out", [S, 128], bf16, kind="ExternalOutput")

    NB = S // QB  # 16 q-blocks

    with tile.TileContext(nc) as tc:
        with (
            tc.tile_pool(name="const", bufs=1) as constp,
            tc.tile_pool(name="big", bufs=1) as bigp,
        ):
            # ---- constants / staged inputs (few big DMAs, 2 queues) ----
            ident = constp.tile([128, 128], bf16)
            nc.sync.dma_start(ident[:], ident_in[:])
            et_sb = bigp.tile([128, EPAD], bf16)
            nc.gpsimd.dma_start(et_sb[:], et[:])
            wp_sb = constp.tile([128, H], bf16)
            nc.sync.dma_start(wp_sb[:], wp[:])

            wqkv_sb = constp.tile([128, 8 * 384], bf16)
            nc.gpsimd.dma_start(
                wqkv_sb[:].rearrange("p (k c) -> p k c", k=8),
                wqkv[:].rearrange("(k p) c -> p k c", p=128))
            xt_sb = bigp.tile([128, 8 * S], bf16)
            for kc in range(8):
                eng = nc.sync if kc % 2 == 0 else nc.gpsimd
                eng.dma_start(xt_sb[:, kc * S:(kc + 1) * S],
                              xT[kc * 128:(kc + 1) * 128, :])

            def xt_c(kc, sl):
                return xt_sb[:, kc * S + sl.start:kc * S + sl.stop]

            def w_c(kc, which):
                o = kc * 384 + which * 128
                return wqkv_sb[:, o:o + 128]

            qT_sb = bigp.tile([128, S], bf16)   # rows 0:64 head0 dims, 64:128 head1
            kT_sb = bigp.tile([128, S], bf16)
            v_t = [bigp.tile([128, 128], bf16, name=f"v{i}") for i in range(NB)]

            # ---- stage 1: qkv^T ----
            with (
                tc.tile_pool(name="ps1", bufs=2, space="PSUM") as ps1,
                tc.tile_pool(name="cp1", bufs=2) as cp1,
            ):
                for st in range(4):  # 512-wide s tiles
                    sl = slice(st * 512, (st + 1) * 512)
                    pq = ps1.tile([128, 512], fp32, name="pq")
                    for kc in range(8):
                        nc.tensor.matmul(pq[:], w_c(kc, 0), xt_c(kc, sl),
                                         start=(kc == 0), stop=(kc == 7))
                    # scale by 1/sqrt(D) during the copy
                    nc.scalar.activation(qT_sb[:, sl], pq[:], AF.Copy, scale=SCALE)
                    pk = ps1.tile([128, 512], fp32, name="pk")
                    for kc in range(8):
                        nc.tensor.matmul(pk[:], w_c(kc, 1), xt_c(kc, sl),
                                         start=(kc == 0), stop=(kc == 7))
                    nc.scalar.activation(kT_sb[:, sl], pk[:], AF.Copy)
                    kf = cp1.tile([128, 512], bf16, name="kf")
                    nc.vector.tensor_copy(kf[:], pk[:])
                    nc.sync.dma_start(kt_out[:, sl], kf[:])
                for mt in range(NB):  # v: [s, d] layout
                    pv = ps1.tile([128, 128], fp32, name="pv")
                    msl = slice(mt * 128, (mt + 1) * 128)
                    for kc in range(8):
                        nc.tensor.matmul(pv[:], xt_c(kc, msl), w_c(kc, 2),
                                         start=(kc == 0), stop=(kc == 7))
                    nc.scalar.activation(v_t[mt][:], pv[:], AF.Copy)
                    vf = cp1.tile([128, 128], bf16, name="vf")
                    nc.vector.tensor_copy(vf[:], pv[:])
                    nc.sync.dma_start(v_out[msl, :], vf[:])

            import bass_rust

            # ---- stage 2: software-pipelined bounce + attention ----
            # Blocks are processed in 4 strided groups {g, g+4, g+8, g+12}
            # (balanced causal work per group).  Within a group the A^T/V
            # matmuls run once per k-tile over all 4 q-blocks (N<=512,
            # right-aligned so causality needs no zero fill) -- 80 av
            # matmuls instead of 272.  P(group) is emitted one group ahead
            # of the attention that consumes it so the DRAM bounce round
            # trip and the DVE-heavy copies overlap PE/ACT work.
            p_drams = {}
            groups = [[g, g + 4, g + 8, g + 12] for g in range(4)]
            with (
                tc.tile_pool(name="dramp", bufs=NB * HPC, space="DRAM") as dramp,
                tc.tile_pool(name="psp0", bufs=1, space="PSUM") as psp0,
                tc.tile_pool(name="sbp0", bufs=3) as sbp0,
                tc.tile_pool(name="pss", bufs=2, space="PSUM") as pss,
                tc.tile_pool(name="psat", bufs=2, space="PSUM") as psat,
                tc.tile_pool(name="psav", bufs=1, space="PSUM") as psav,
                tc.tile_pool(name="sb2", bufs=2) as sb2,
                tc.tile_pool(name="sb3", bufs=4) as sb3,
                tc.tile_pool(name="sba", bufs=18) as sba,
            ):
                def emit_p(blk):
                    n0 = blk * QB
                    W = n0 + 256
                    mlo = S - 128 - n0
                    for h in range(HPC):
                        hsl = slice(h * 64, (h + 1) * 64)
                        p_sb = sbp0.tile([128, (W + 511) // 512 * 512], bf16,
                                         name="p_sb")
                        for wc in range(0, W, 512):
                            ww = min(512, W - wc)
                            pp = psp0.tile([128, 512], fp32, name="pp")
                            nc.tensor.matmul(
                                pp[:, 0:ww],
                                qT_sb[hsl, n0:n0 + 128],
                                et_sb[hsl, mlo + wc:mlo + wc + ww],
                                start=True, stop=True)
                            nc.vector.tensor_copy(p_sb[:, wc:wc + ww],
                                                  pp[:, 0:ww])
                        # last 128 cols of P are read only by causally
                        # masked positions: bake the -1e9 mask in here
                        nc.gpsimd.memset(p_sb[:, W - 128:W], NEG)
                        p_dram = dramp.tile([128, W], bf16, name=f"pd{blk}_{h}")
                        nc.gpsimd.dma_start(p_dram[:], p_sb[:, 0:W])
                        p_drams[(blk, h)] = (p_dram, W)

                def emit_scores(blk, a_t, recips):
                    n0 = blk * QB
                    kw = n0 + 128         # causal scores width
                    npass = (kw + 1023) // 1024
                    sums = sb2.tile([128, 4], fp32, name="sums")
                    recip = sb2.tile([128, 2], fp32, name="recip")
                    for h in range(HPC):
                        hsl = slice(h * 64, (h + 1) * 64)
                        p_dram, W = p_drams[(blk, h)]
                        a_sb = sba.tile([128, S], bf16, name="a_sb")
                        rel_sb = sb3.tile([128, S], bf16, name="rel_sb")
                        diag = bass_rust.AP(
                            tensor=p_dram.tensor,
                            offset=p_dram.offset + 127,
                            ap=[[W - 1, 128], [1, kw]],
                        )
                        nc.gpsimd.dma_start(rel_sb[:, 0:kw], diag)
                        for ip in range(npass):
                            pb = ip * 1024
                            pw = min(1024, kw - pb)
                            ps_s = pss.tile([128, 1024], fp32, name="ps_s")
                            for kc in range(pb, pb + pw, 512):
                                ww = min(512, pb + pw - kc)
                                nc.tensor.matmul(
                                    ps_s[:, kc - pb:kc - pb + ww],
                                    qT_sb[hsl, n0:n0 + 128],
                                    kT_sb[hsl, kc:kc + ww],
                                    start=True, stop=False)
                                nc.tensor.matmul(
                                    ps_s[:, kc - pb:kc - pb + ww], ident[:],
                                    rel_sb[:, kc:kc + ww],
                                    start=False, stop=True,
                                    skip_group_check=True)
                            nc.scalar.activation(
                                a_sb[:, pb:pb + pw], ps_s[:, 0:pw], AF.Exp,
                                accum_out=sums[:, h * npass + ip:
                                               h * npass + ip + 1])
                        if npass > 1:
                            ssum = sb2.tile([128, 2], fp32, name="ssum")
                            nc.vector.tensor_reduce(
                                ssum[:, h:h + 1],
                                sums[:, h * npass:(h + 1) * npass],
                                axis=mybir.AxisListType.X,
                                op=mybir.AluOpType.add)
                            nc.vector.reciprocal(recip[:, h:h + 1],
                                                 ssum[:, h:h + 1])
                        else:
                            nc.vector.reciprocal(recip[:, h:h + 1],
                                                 sums[:, h * npass:
                                                      h * npass + 1])
                        nc.vector.tensor_scalar_mul(a_sb[:, 0:kw],
                                                    a_sb[:, 0:kw],
                                                    recip[:, h:h + 1])
                        a_t[(blk, h)] = a_sb

                def emit_av_proj(g, a_t):
                    blks = groups[g]
                    nkt = blks[-1] + 1
                    ps_av = psav.tile([128, 512], fp32, name="ps_av")
                    for h in range(HPC):
                        hsl = slice(h * 64, (h + 1) * 64)
                        for kt in range(nkt):
                            jmin = max(0, -(-(kt - g) // 4))  # ceil
                            N = (4 - jmin) * 128
                            ps_at = psat.tile([128, 512], bf16, name="ps_at")
                            for j in range(jmin, 4):
                                nc.tensor.transpose(
                                    ps_at[:, (j - jmin) * 128:
                                          (j - jmin + 1) * 128],
                                    a_t[(blks[j], h)][:, kt * 128:
                                                      (kt + 1) * 128],
                                    ident[:])
                            at_sb = sb3.tile([128, 512], bf16, name="at_sb")
                            if kt % 2 == 0:
                                nc.vector.tensor_copy(at_sb[:, 0:N],
                                                      ps_at[:, 0:N])
                            else:
                                nc.scalar.activation(at_sb[:, 0:N],
                                                     ps_at[:, 0:N], AF.Copy)
                            nc.tensor.matmul(
                                ps_av[hsl, jmin * 128:512], v_t[kt][:, hsl],
                                at_sb[:, 0:N],
                                start=(kt == 0), stop=(kt == nkt - 1),
                                skip_group_check=True)
                    avT_sb = sb2.tile([128, 512], bf16, name="avT_sb")
                    nc.vector.tensor_copy(avT_sb[:], ps_av[:])
                    # --- projection: 4 q-blocks of this group ---
                    for j, blk in enumerate(blks):
                        n0 = blk * QB
                        o_sb = sb2.tile([128, H], bf16, name="o_sb")
                        for nt in range(2):
                            nsl = slice(nt * 512, (nt + 1) * 512)
                            ps_o = psp0.tile([128, 512], fp32, name="pp")
                            nc.tensor.matmul(
                                ps_o[:], avT_sb[:, j * 128:(j + 1) * 128],
                                wp_sb[:, nsl], start=True, stop=True)
                            nc.vector.tensor_copy(o_sb[:, nsl], ps_o[:])
                        nc.sync.dma_start(out_p[n0:n0 + 128, :], o_sb[:])

                a_t = {}
                for step in range(6):
                    if step < 4:
                        for blk in groups[step]:
                            emit_p(blk)
                    if 1 <= step < 5:
                        for blk in groups[step - 1]:
                            emit_scores(blk, a_t, None)
                    if step >= 2:
                        emit_av_proj(step - 2, a_t)

    nc.compile()
    return nc


def build_in_maps(inputs):
    x = np.asarray(inputs["x"], dtype=np.float32)
    W_attn = np.asarray(inputs["W_attn"], dtype=np.float32)
    W_proj = np.asarray(inputs["W_proj"], dtype=np.float32)
    E = np.asarray(inputs["E"], dtype=np.float32)

    xT = np.ascontiguousarray(x.reshape(S, H).T).astype(BF16)
    ident = np.eye(128, dtype=BF16)
    in_maps = []
    for c in range(NCORES):
        h0 = HPC * c
        col = slice(h0 * D, h0 * D + HPC * D)
        etc = np.zeros((128, EPAD), dtype=BF16)
        for hh in range(HPC):
            etc[hh * 64:(hh + 1) * 64, :S] = E[h0 + hh].T.astype(BF16)
        wqkv = np.concatenate([
            W_attn[:, col], W_attn[:, H:][:, col], W_attn[:, 2 * H:][:, col]],
            axis=1)
        in_maps.append({
            "xT": xT,
            "wqkv": np.ascontiguousarray(wqkv).astype(BF16),
            "et": etc,
            "wp": np.ascontiguousarray(W_proj[col, :]).astype(BF16),
            "ident": ident,
        })
    return in_maps


def kernel(x, mask, W_attn, b_attn, W_proj, b_proj, E):
    from concourse.bass_utils import run_bass_kernel_spmd

    b_proj = np.asarray(b_proj, dtype=np.float32)
    in_maps = build_in_maps(dict(x=x, W_attn=W_attn, W_proj=W_proj, E=E))

    nc = _build_graph()
    res = run_bass_kernel_spmd(nc, in_maps, core_ids=list(range(NCORES)))
    results = res.results

    out = np.zeros((S, H), dtype=np.float32)
    k_full = np.zeros((NH, S, D), dtype=np.float32)
    v_full = np.zeros((NH, S, D), dtype=np.float32)
    for c in range(NCORES):
        out += results[c]["out_p"].astype(np.float32)
        for hh in range(HPC):
            h = HPC * c + hh
            k_full[h] = results[c]["kt_out"][hh * 64:(hh + 1) * 64, :].T.astype(np.float32)
            v_full[h] = results[c]["v_out"][:, hh * 64:(hh + 1) * 64].astype(np.float32)
    out = out + b_proj.reshape(1, H)
    out = out.reshape(1, S, H)
    present = np.stack([k_full, v_full], axis=0)[None]  # [1, 2, NH, S, D]
    return out, present


# revision 31
# speedup vs baseline: 1.2353x; 1.2353x over previous
"""Distributed Trainium2 kernel for causal attention with relative position
embeddings (Music-Transformer skew), TP over heads: 8 cores x 2 heads.

Reference computation (B=1, S=2048, H=1024, NH=16, D=64):
  qkv = x @ W_attn + b_attn ; q,k,v split into heads
  w   = (q k^T + skew(q E^T)) / sqrt(D) + causal_mask * -1e9
  a   = softmax(w); out = (a v, merged) @ W_proj + b_proj
  present = stack([k, v])

Sharding: core c owns heads (2c, 2c+1). Everything head-local on device;
host sums the 8 partial projection outputs (the "all-reduce") and
assembles `present` from per-core k/v shards.

Skew trick on device: rel[n, j] = q[n] . E[s-1-n+j] (valid j<=n).  Per
128-row q-block at n0 we matmul P[r, w] = q[n0+r] . E[mlo+w] with
mlo = S-128-n0 over a window W = n0+256 (E zero-padded to 2176 cols so
masked/out-of-range reads are in-bounds), write P to a DRAM bounce
buffer, and read it back through a diagonal access pattern
addr = (127+kc) + r*(W-1) + c which lands rel[r, kc+c] exactly.  The
rel tile is then accumulated into the scores PSUM with an
identity-weight matmul.  The causal mask is free: the last 128 columns
of each P buffer are read exclusively by masked positions, so they are
memset to -1e9 before the bounce.  exp() runs on ScalarE with
accum_out producing the softmax denominators for free; rows of A are
normalized in place with a per-partition scalar multiply.  A^T tiles
(TensorE transpose) feed (A^T)'V matmuls that run once per k-tile
across 4 strided q-blocks (N<=512, right-aligned), and the projection
consumes the avT accumulator directly.
"""

import numpy as np
import ml_dtypes

S = 2048
H = 1024
NH = 16
D = 64
HPC = 2          # heads per core
NCORES = 8
QB = 128         # q-block rows
EPAD = 2176      # padded E length: max m index = 2174
SCALE = 1.0 / 8.0  # 1/sqrt(D)
NEG = -1.0e9

BF16 = ml_dtypes.bfloat16


def _build_graph():
    import concourse.bacc as bacc
    import concourse.bass as bass
    import concourse.mybir as mybir
    import concourse.tile as tile

    fp32 = mybir.dt.float32
    bf16 = mybir.dt.bfloat16
    AF = mybir.ActivationFunctionType

    nc = bacc.Bacc(
        "TRN2",
        target_bir_lowering=False,
        debug=False,
        enable_asserts=False,
        num_devices=NCORES,
    )

    # ---- I/O ----
    xT = nc.dram_tensor("xT", [H, S], bf16, kind="ExternalInput")
    wqkv = nc.dram_tensor("wqkv", [H, 384], bf16, kind="ExternalInput")
    et = nc.dram_tensor("et", [128, EPAD], bf16, kind="ExternalInput")
    wp = nc.dram_tensor("wp", [128, H], bf16, kind="ExternalInput")
    ident_in = nc.dram_tensor("ident", [128, 128], bf16, kind="ExternalInput")

    out_p = nc.dram_tensor("out_p", [S, H], bf16, kind="ExternalOutput")
    kt_out = nc.dram_tensor("kt_out", [128, S], bf16, kind="ExternalOutput")
    v_out = nc.dram_tensor("v_out", [128, S], bf16, kind="ExternalOutput")

    NB = S // QB  # 16 q-blocks

    with tile.TileContext(nc) as tc:
        with (
            tc.tile_pool(name="const", bufs=1) as constp,
            tc.tile_pool(name="big", bufs=1) as bigp,
        ):
            # ---- constants / staged inputs (few big DMAs, 2 queues) ----
            ident = constp.tile([128, 128], bf16)
            nc.sync.dma_start(ident[:], ident_in[:])
            et_sb = bigp.tile([128, EPAD], bf16)
            nc.gpsimd.dma_start(et_sb[:], et[:])
            wp_sb = constp.tile([128, H], bf16)
            nc.sync.dma_start(wp_sb[:], wp[:])

            wqkv_sb = constp.tile([128, 8 * 384], bf16)
            nc.gpsimd.dma_start(
                wqkv_sb[:].rearrange("p (k c) -> p k c", k=8),
                wqkv[:].rearrange("(k p) c -> p k c", p=128))
            xt_sb = bigp.tile([128, 8 * S], bf16)
            for kc in range(8):
                eng = nc.sync if kc % 2 == 0 else nc.gpsimd
                eng.dma_start(xt_sb[:, kc * S:(kc + 1) * S],
                              xT[kc * 128:(kc + 1) * 128, :])

            def xt_c(kc, sl):
                return xt_sb[:, kc * S + sl.start:kc * S + sl.stop]

            def w_c(kc, which):
                o = kc * 384 + which * 128
                return wqkv_sb[:, o:o + 128]

            qT_sb = bigp.tile([128, S], bf16)   # rows 0:64 head0 dims, 64:128 head1
            kT_sb = bigp.tile([128, S], bf16)
            v_t = [bigp.tile([128, 128], bf16, name=f"v{i}") for i in range(NB)]

            import bass_rust

            # ---- fused qkv + bounce + attention pipeline ----
            # Emission order interleaves the skew bounce P(g) (needs only
            # qT) into the qkv stage, then runs a 3-deep software pipeline
            # P(g+2) / scores(g+1) / av+proj(g) over 4 strided block
            # groups {g, g+4, g+8, g+12} (balanced causal work).  The A^T/V
            # matmuls run once per k-tile over all 4 q-blocks of a group
            # (N<=512, right-aligned: causality needs no zero fill).
            p_drams = {}
            groups = [[g, g + 4, g + 8, g + 12] for g in range(4)]
            with (
                tc.tile_pool(name="dramp", bufs=NB * HPC, space="DRAM") as dramp,
                tc.tile_pool(name="psp0", bufs=2, space="PSUM") as psp0,
                tc.tile_pool(name="sbp0", bufs=4) as sbp0,
                tc.tile_pool(name="cp1", bufs=3) as cp1,
                tc.tile_pool(name="pss", bufs=3, space="PSUM") as pss,
                tc.tile_pool(name="psat", bufs=2, space="PSUM") as psat,
                tc.tile_pool(name="psav", bufs=1, space="PSUM") as psav,
                tc.tile_pool(name="sb2", bufs=4) as sb2,
                tc.tile_pool(name="sb3", bufs=6) as sb3,
                tc.tile_pool(name="sba", bufs=20) as sba,
            ):
                def emit_qt():
                    for st in range(4):
                        sl = slice(st * 512, (st + 1) * 512)
                        pq = psp0.tile([128, 512], fp32, name="pp")
                        for kc in range(8):
                            nc.tensor.matmul(pq[:], w_c(kc, 0), xt_c(kc, sl),
                                             start=(kc == 0), stop=(kc == 7))
                        # fold the 1/sqrt(D) scale into Q
                        nc.scalar.activation(qT_sb[:, sl], pq[:], AF.Copy,
                                             scale=SCALE)

                def emit_kt():
                    for st in range(4):
                        sl = slice(st * 512, (st + 1) * 512)
                        pk = psp0.tile([128, 512], fp32, name="pp")
                        for kc in range(8):
                            nc.tensor.matmul(pk[:], w_c(kc, 1), xt_c(kc, sl),
                                             start=(kc == 0), stop=(kc == 7))
                        nc.scalar.activation(kT_sb[:, sl], pk[:], AF.Copy)
                        kf = cp1.tile([128, 512], bf16, name="kf")
                        nc.vector.tensor_copy(kf[:], pk[:])
                        nc.sync.dma_start(kt_out[:, sl], kf[:])

                def emit_v():
                    # vT like kT (wide-N matmuls), then 16 PE transposes
                    # give the [s, d] tiles the A^T V stage consumes.
                    vT_sb = bigp.tile([128, S], bf16)
                    for st in range(4):
                        sl = slice(st * 512, (st + 1) * 512)
                        pvt = psp0.tile([128, 512], fp32, name="pp")
                        for kc in range(8):
                            nc.tensor.matmul(pvt[:], w_c(kc, 2), xt_c(kc, sl),
                                             start=(kc == 0), stop=(kc == 7))
                        nc.scalar.activation(vT_sb[:, sl], pvt[:], AF.Copy)
                        vf = cp1.tile([128, 512], bf16, name="kf")
                        nc.vector.tensor_copy(vf[:], pvt[:])
                        nc.sync.dma_start(v_out[:, sl], vf[:])
                    for mt in range(NB):
                        pvi = psat.tile([128, 512], bf16, name="ps_at")
                        nc.tensor.transpose(
                            pvi[:, 0:128],
                            vT_sb[:, mt * 128:(mt + 1) * 128], ident[:])
                        nc.vector.tensor_copy(v_t[mt][:], pvi[:, 0:128])

                def emit_p(blk, act_split=False):
                    n0 = blk * QB
                    W = n0 + 256
                    mlo = S - 128 - n0
                    for h in range(HPC):
                        hsl = slice(h * 64, (h + 1) * 64)
                        p_sb = sbp0.tile([128, (W + 511) // 512 * 512], bf16,
                                         name="p_sb")
                        for wc in range(0, W, 512):
                            ww = min(512, W - wc)
                            pp = psp0.tile([128, 512], fp32, name="pp")
                            nc.tensor.matmul(
                                pp[:, 0:ww],
                                qT_sb[hsl, n0:n0 + 128],
                                et_sb[hsl, mlo + wc:mlo + wc + ww],
                                start=True, stop=True)
                            if act_split and (wc // 512) % 2 == 1:
                                nc.scalar.activation(p_sb[:, wc:wc + ww],
                                                     pp[:, 0:ww], AF.Copy)
                            else:
                                nc.vector.tensor_copy(p_sb[:, wc:wc + ww],
                                                      pp[:, 0:ww])
                        # last 128 cols of P are read only by causally
                        # masked positions: bake the -1e9 mask in here
                        nc.gpsimd.memset(p_sb[:, W - 128:W], NEG)
                        p_dram = dramp.tile([128, W], bf16, name=f"pd{blk}_{h}")
                        nc.gpsimd.dma_start(p_dram[:], p_sb[:, 0:W])
                        p_drams[(blk, h)] = (p_dram, W)

                def emit_scores(blk, a_t, recips):
                    n0 = blk * QB
                    kw = n0 + 128         # causal scores width
                    npass = (kw + 511) // 512
                    sums = sb2.tile([128, 8], fp32, name="sums")
                    recip = sb2.tile([128, 2], fp32, name="recip")
                    for h in range(HPC):
                        hsl = slice(h * 64, (h + 1) * 64)
                        p_dram, W = p_drams[(blk, h)]
                        a_sb = sba.tile([128, S], bf16, name="a_sb")
                        rel_sb = sb3.tile([128, S], bf16, name="rel_sb")
                        diag = bass_rust.AP(
                            tensor=p_dram.tensor,
                            offset=p_dram.offset + 127,
                            ap=[[W - 1, 128], [1, kw]],
                        )
                        nc.gpsimd.dma_start(rel_sb[:, 0:kw], diag)
                        for ip in range(npass):
                            pb = ip * 512
                            pw = min(512, kw - pb)
                            ps_s = pss.tile([128, 512], fp32, name="ps_s")
                            nc.tensor.matmul(
                                ps_s[:, 0:pw],
                                qT_sb[hsl, n0:n0 + 128],
                                kT_sb[hsl, pb:pb + pw],
                                start=True, stop=False)
                            nc.tensor.matmul(
                                ps_s[:, 0:pw], ident[:],
                                rel_sb[:, pb:pb + pw],
                                start=False, stop=True,
                                skip_group_check=True)
                            nc.scalar.activation(
                                a_sb[:, pb:pb + pw], ps_s[:, 0:pw], AF.Exp,
                                accum_out=sums[:, h * 4 + ip:h * 4 + ip + 1])
                        if npass > 1:
                            ssum = sb2.tile([128, 2], fp32, name="ssum")
                            nc.vector.tensor_reduce(
                                ssum[:, h:h + 1],
                                sums[:, h * 4:h * 4 + npass],
                                axis=mybir.AxisListType.X,
                                op=mybir.AluOpType.add)
                            nc.vector.reciprocal(recip[:, h:h + 1],
                                                 ssum[:, h:h + 1])
                        else:
                            nc.vector.reciprocal(recip[:, h:h + 1],
                                                 sums[:, h * 4:h * 4 + 1])
                        nc.vector.tensor_scalar_mul(a_sb[:, 0:kw],
                                                    a_sb[:, 0:kw],
                                                    recip[:, h:h + 1])
                        a_t[(blk, h)] = a_sb

                def emit_av_proj(g, a_t):
                    # k-tile-major over all 4 q-blocks; each q-block's
                    # projection is emitted the moment its final k-tile
                    # lands (its avT columns are complete then).
                    blks = groups[g]
                    nkt = blks[-1] + 1
                    ps_av = psav.tile([128, 512], fp32, name="ps_av")
                    for kt in range(nkt):
                        jmin = max(0, -(-(kt - g) // 4))  # ceil
                        N = (4 - jmin) * 128
                        for h in range(HPC):
                            hsl = slice(h * 64, (h + 1) * 64)
                            ps_at = psat.tile([128, 512], bf16, name="ps_at")
                            for j in range(jmin, 4):
                                nc.tensor.transpose(
                                    ps_at[:, (j - jmin) * 128:
                                          (j - jmin + 1) * 128],
                                    a_t[(blks[j], h)][:, kt * 128:
                                                      (kt + 1) * 128],
                                    ident[:])
                            at_sb = sb3.tile([128, 512], bf16, name="at_sb")
                            if kt % 2 == 0:
                                nc.vector.tensor_copy(at_sb[:, 0:N],
                                                      ps_at[:, 0:N])
                            else:
                                nc.scalar.activation(at_sb[:, 0:N],
                                                     ps_at[:, 0:N], AF.Copy)
                            nc.tensor.matmul(
                                ps_av[hsl, jmin * 128:512], v_t[kt][:, hsl],
                                at_sb[:, 0:N],
                                start=(kt == 0), stop=(kt == nkt - 1),
                                skip_group_check=True)
                        if kt in blks:
                            j = blks.index(kt)
                            jsl = slice(j * 128, (j + 1) * 128)
                            avT_sb = sb2.tile([128, 128], bf16, name="avT_sb")
                            nc.vector.tensor_copy(avT_sb[:], ps_av[:, jsl])
                            n0 = kt * QB
                            o_sb = sb2.tile([128, H], bf16, name="o_sb")
                            for nt in range(2):
                                nsl = slice(nt * 512, (nt + 1) * 512)
                                ps_o = psp0.tile([128, 512], fp32, name="pp")
                                nc.tensor.matmul(
                                    ps_o[:], avT_sb[:], wp_sb[:, nsl],
                                    start=True, stop=True)
                                nc.vector.tensor_copy(o_sb[:, nsl], ps_o[:])
                            nc.sync.dma_start(out_p[n0:n0 + 128, :], o_sb[:])

                a_t = {}
                emit_qt()
                for blk in groups[0]:
                    emit_p(blk, act_split=True)
                emit_kt()
                for blk in groups[1]:
                    emit_p(blk, act_split=True)
                emit_v()
                for step in range(2, 6):
                    if step < 4:
                        for blk in groups[step]:
                            emit_p(blk)
                    for blk in groups[step - 2]:
                        emit_scores(blk, a_t, None)
                    if step >= 3:
                        emit_av_proj(step - 3, a_t)
                emit_av_proj(3, a_t)

    nc.compile()
    return nc


def build_in_maps(inputs):
    x = np.asarray(inputs["x"], dtype=np.float32)
    W_attn = np.asarray(inputs["W_attn"], dtype=np.float32)
    W_proj = np.asarray(inputs["W_proj"], dtype=np.float32)
    E = np.asarray(inputs["E"], dtype=np.float32)

    xT = np.ascontiguousarray(x.reshape(S, H).T).astype(BF16)
    ident = np.eye(128, dtype=BF16)
    in_maps = []
    for c in range(NCORES):
        h0 = HPC * c
        col = slice(h0 * D, h0 * D + HPC * D)
        etc = np.zeros((128, EPAD), dtype=BF16)
        for hh in range(HPC):
            etc[hh * 64:(hh + 1) * 64, :S] = E[h0 + hh].T.astype(BF16)
        wqkv = np.concatenate([
            W_attn[:, col], W_attn[:, H:][:, col], W_attn[:, 2 * H:][:, col]],
            axis=1)
        in_maps.append({
            "xT": xT,
            "wqkv": np.ascontiguousarray(wqkv).astype(BF16),
            "et": etc,
            "wp": np.ascontiguousarray(W_proj[col, :]).astype(BF16),
            "ident": ident,
        })
    return in_maps


def kernel(x, mask, W_attn, b_attn, W_proj, b_proj, E):
    from concourse.bass_utils import run_bass_kernel_spmd

    b_proj = np.asarray(b_proj, dtype=np.float32)
    in_maps = build_in_maps(dict(x=x, W_attn=W_attn, W_proj=W_proj, E=E))

    nc = _build_graph()
    res = run_bass_kernel_spmd(nc, in_maps, core_ids=list(range(NCORES)))
    results = res.results

    out = np.zeros((S, H), dtype=np.float32)
    k_full = np.zeros((NH, S, D), dtype=np.float32)
    v_full = np.zeros((NH, S, D), dtype=np.float32)
    for c in range(NCORES):
        out += results[c]["out_p"].astype(np.float32)
        for hh in range(HPC):
            h = HPC * c + hh
            k_full[h] = results[c]["kt_out"][hh * 64:(hh + 1) * 64, :].T.astype(np.float32)
            v_full[h] = results[c]["v_out"][hh * 64:(hh + 1) * 64, :].T.astype(np.float32)
    out = out + b_proj.reshape(1, H)
    out = out.reshape(1, S, H)
    present = np.stack([k_full, v_full], axis=0)[None]  # [1, 2, NH, S, D]
    return out, present
